# revision 22
# baseline (speedup 1.0000x reference)
"""CrossCosineEmbeddingLoss kernel for 8 trn2 NeuronCores (v6).

loss = mean over all (i,j) of: 1 - cos(x_i, y_j) if i==j else relu(cos(x_i, y_j))

Identity: total = sum_ij relu(xhat_i . y_j) * rny_j
                + sum_i (1 - sim_ii - relu(sim_ii))
(y unnormalized on device; 1/||y_j|| applied per j after the i-sum since
relu(c*s) = c*relu(s) for c>0; the n-sized diagonal correction and the
1/||y|| scaling are applied on the host in fp64.)

Sharding: 8 cores = 4 x-shards (2048 rows) x 2 j-halves (4096 cols).
Per-core: 32 j-tiles, blocks of [128 j, 2048 i] fp32 PSUM (4 banks, bufs=2).

Per-core pipeline:
  - x shard fp32: DVE sumsq -> rsqrt -> DVE scale (bf16 out) -> PE transpose
    (bf16) -> DVE copy to xhatT
  - y: host passes ybT (transposed, bf16) j-half; yT tiles are plain DMA loads
  - main: 32 blocks: 4 bf16 matmuls -> [128,2048] fp32 PSUM -> one
    relu+accum instruction into R[:, t], alternating ACT / DVE
  - out = R [128, 32]; host: rny scaling, diagonal, final mean.
"""

import numpy as np
import ml_dtypes

import concourse.bacc as bacc
import concourse.bass as bass
import concourse.tile as tile
from concourse import mybir
from concourse.bass_utils import run_bass_kernel_spmd
from concourse.masks import make_identity

N, D = 8192, 128
NCORES = 8
XSH = 2048                # x rows per shard
JSH = 4096                # j columns per half
TX = XSH // 128           # 16 x-tiles per core
TJ = JSH // 128           # 32 j-tiles per core

f32 = mybir.dt.float32
bf16 = mybir.dt.bfloat16
AF = mybir.ActivationFunctionType
ALU = mybir.AluOpType

# main-loop reducer assignment: weighted round-robin over ACT / DVE
_COUNTS = {"act": 17, "dve": 15}


def _assignment():
    quota = dict(_COUNTS)
    total = sum(quota.values())
    acc = {k: 0.0 for k in quota}
    out = []
    for _ in range(total):
        for k in quota:
            acc[k] += quota[k] / total
        pick = max(acc, key=lambda k: acc[k])
        acc[pick] -= 1.0
        out.append(pick)
    return out


ASSIGN = _assignment()

_CACHE = {}


def _build():
    if "nc" in _CACHE:
        return _CACHE["nc"]
    nc = bacc.Bacc("TRN2", target_bir_lowering=False, debug=False,
                   num_devices=NCORES)
    xs_d = nc.dram_tensor("xs", [XSH, D], f32, kind="ExternalInput")
    ybt_d = nc.dram_tensor("ybt", [D, JSH], bf16, kind="ExternalInput")
    out_d = nc.dram_tensor("out", [128, TJ], f32, kind="ExternalOutput")

    with tile.TileContext(nc) as tc:
        with (
            tc.tile_pool(name="singles", bufs=1) as singles,
            tc.tile_pool(name="scr", bufs=2) as scr,
        ):
            ident = singles.tile([128, 128], bf16)
            make_identity(nc, ident[:])

            xnat = singles.tile([128, TX, 128], f32)
            xhat = singles.tile([128, TX, 128], bf16)
            xhatT = singles.tile([128, TX, 128], bf16)
            yT = singles.tile([128, TJ, 128], bf16)
            nx2 = singles.tile([128, TX], f32)
            t1x = singles.tile([128, TX], f32)
            rnx = singles.tile([128, TX], f32)
            R = singles.tile([128, TJ], f32)

            # ---- input DMAs: x shard per-tile (fast head), y-half groups
            for t in range(TX):
                nc.sync.dma_start(
                    out=xnat[:, t, :], in_=xs_d[128 * t:128 * (t + 1), :])
            for g in range(TJ // 8):
                nc.sync.dma_start(
                    out=yT[:, 8 * g:8 * (g + 1), :],
                    in_=ybt_d[:, 1024 * g:1024 * (g + 1)]
                    .rearrange("p (a b) -> p a b", b=128))

            # ---- x norms + scale to bf16 (DVE)
            for t in range(TX):
                nc.vector.scalar_tensor_tensor(
                    out=scr.tile([128, 128], f32, tag='sd', name='sd')[:],
                    in0=xnat[:, t, :], scalar=1.0, in1=xnat[:, t, :],
                    op0=ALU.mult, op1=ALU.mult, accum_out=nx2[:, t:t + 1])
            nc.vector.reciprocal(t1x[:], nx2[:])
            nc.scalar.sqrt(rnx[:], t1x[:])   # 1/||x_r||
            for t in range(TX):
                nc.vector.tensor_scalar(
                    out=xhat[:, t, :], in0=xnat[:, t, :],
                    scalar1=rnx[:, t:t + 1], scalar2=None, op0=ALU.mult)

            # ---- x transpose on PE (bf16), DVE copyback
            flat = xhatT[:].rearrange("p a b -> p (a b)")
            with tc.tile_pool(name="tpsum", bufs=1, space="PSUM") as tpsum:
                ptx = tpsum.tile([128, TX * 128], bf16, tag="tp")
                for t in range(TX):
                    nc.tensor.transpose(ptx[:, 128 * t:128 * (t + 1)],
                                        xhat[:, t, :], ident[:])
                nc.vector.tensor_copy(out=flat[:, :1024], in_=ptx[:, :1024])
                nc.vector.tensor_copy(out=flat[:, 1024:], in_=ptx[:, 1024:])

            # ---- main: per j-block bf16 matmuls (fp32 PSUM, 4 banks) +
            # one relu+accum per block, alternating ACT / DVE
            with tc.tile_pool(name="mpsum", bufs=2, space="PSUM") as mpsum:
                for t in range(TJ):
                    ps = mpsum.tile([128, 2048], f32, tag="mp")
                    lhsT = yT[:, t, :]
                    for q in range(4):
                        nc.tensor.matmul(ps[:, 512 * q:512 * (q + 1)],
                                         lhsT, flat[:, 512 * q:512 * (q + 1)])
                    if ASSIGN[t] == "act":
                        nc.scalar.activation(
                            ps[:], ps[:], AF.Relu, accum_out=R[:, t:t + 1])
                    else:
                        nc.vector.tensor_scalar(
                            out=ps[:], in0=ps[:], scalar1=0.0, scalar2=None,
                            op0=ALU.max, op1=ALU.add,
                            accum_out=R[:, t:t + 1])

            # ---- output
            nc.sync.dma_start(out=out_d[:], in_=R[:])

    nc.compile()
    _CACHE["nc"] = nc
    return nc


def _in_maps(x, y):
    yb = y.astype(ml_dtypes.bfloat16)
    ybt = np.ascontiguousarray(yb.T)          # [D, N]
    maps = []
    for c in range(NCORES):
        s, jh = c // 2, c % 2
        maps.append({
            "xs": np.ascontiguousarray(x[XSH * s:XSH * (s + 1)]),
            "ybt": np.ascontiguousarray(ybt[:, JSH * jh:JSH * (jh + 1)]),
        })
    return maps


def _combine(results, x, y):
    x64 = x.astype(np.float64)
    y64 = y.astype(np.float64)
    ny = np.sqrt((y64 ** 2).sum(axis=1))
    rny = 1.0 / np.maximum(ny, 1e-8)          # [N]
    total = 0.0
    for c in range(NCORES):
        jh = c % 2
        R = results[c]["out"].astype(np.float64)      # [128, TJ]
        j = JSH * jh + 128 * np.arange(TJ)[None, :] + np.arange(128)[:, None]
        total += (R * rny[j]).sum()
    # diagonal correction in fp64 on host (n of n^2 terms)
    nx = np.sqrt((x64 ** 2).sum(axis=1))
    sim_d = (x64 * y64).sum(axis=1) / np.maximum(nx * ny, 1e-8)
    total += (1.0 - sim_d - np.maximum(sim_d, 0.0)).sum()
    return np.float32(total / (float(N) * float(N)))


def _run(x, y, trace=False):
    nc = _build()
    res = run_bass_kernel_spmd(nc, _in_maps(x, y), list(range(NCORES)),
                               trace=trace)
    return _combine(res.results, x, y), res


def kernel(x, y):
    x = np.asarray(x, dtype=np.float32)
    y = np.asarray(y, dtype=np.float32)
    loss, _ = _run(x, y, trace=False)
    return loss


# revision 23
# speedup vs baseline: 1.3846x; 1.3846x over previous
"""CrossCosineEmbeddingLoss kernel for 8 trn2 NeuronCores (v7).

loss = mean over all (i,j) of: 1 - cos(x_i, y_j) if i==j else relu(cos(x_i, y_j))

Identity: total = sum_ij relu(xhat_i . y_j) * rny_j
                + sum_i (1 - sim_ii - relu(sim_ii))
(y unnormalized on device; 1/||y_j|| applied per j after the i-sum since
relu(c*s) = c*relu(s) for c>0; the n-sized diagonal correction and the
1/||y|| scaling run on the host in fp64.)

Sharding: rows of x across 8 cores (1024 each); y replicated, passed from the
host already transposed + bf16 (ybT [D, N]).

Per-core pipeline:
  - x shard fp32: DVE sumsq -> rsqrt -> DVE scale (bf16 out) -> PE transpose
    (bf16) -> DVE copy to xhatT
  - warm-up: 12 filler matmuls flip the PE HAM clock gate to 2.4 GHz early
  - main: 64 j-blocks: 2 bf16 matmuls -> [128,1024] fp32 PSUM (2 banks,
    bufs=4 = 4-deep pipeline) -> one relu+accum into R[:, t], ACT/DVE split
  - out = R [128, 64]; host: rny scaling, diagonal, final mean.
"""

import numpy as np
import ml_dtypes

import concourse.bacc as bacc
import concourse.bass as bass
import concourse.tile as tile
from concourse import mybir
from concourse.bass_utils import run_bass_kernel_spmd
from concourse.masks import make_identity

N, D = 8192, 128
NCORES = 8
SH = N // NCORES          # 1024 rows of x per core
TX = SH // 128            # 8 x-tiles per core
TY = N // 128             # 64 y-tiles

f32 = mybir.dt.float32
bf16 = mybir.dt.bfloat16
AF = mybir.ActivationFunctionType
ALU = mybir.AluOpType

# main-loop reducer assignment: weighted round-robin over ACT / DVE
_COUNTS = {"act": 32, "dve": 32}


def _assignment():
    quota = dict(_COUNTS)
    total = sum(quota.values())
    acc = {k: 0.0 for k in quota}
    out = []
    for _ in range(total):
        for k in quota:
            acc[k] += quota[k] / total
        pick = max(acc, key=lambda k: acc[k])
        acc[pick] -= 1.0
        out.append(pick)
    return out


ASSIGN = _assignment()

_CACHE = {}


def _build():
    if "nc" in _CACHE:
        return _CACHE["nc"]
    nc = bacc.Bacc("TRN2", target_bir_lowering=False, debug=False,
                   num_devices=NCORES)
    xs_d = nc.dram_tensor("xs", [SH, D], f32, kind="ExternalInput")
    ybt_d = nc.dram_tensor("ybt", [D, N], bf16, kind="ExternalInput")
    out_d = nc.dram_tensor("out", [128, TY], f32, kind="ExternalOutput")

    with tile.TileContext(nc) as tc:
        with (
            tc.tile_pool(name="singles", bufs=1) as singles,
            tc.tile_pool(name="scr", bufs=2) as scr,
        ):
            ident = singles.tile([128, 128], bf16)
            make_identity(nc, ident[:])

            xnat = singles.tile([128, TX, 128], f32)
            xhat = singles.tile([128, TX, 128], bf16)
            xhatT = singles.tile([128, TX, 128], bf16)
            yT = singles.tile([128, TY, 128], bf16)
            nx2 = singles.tile([128, TX], f32)
            t1x = singles.tile([128, TX], f32)
            rnx = singles.tile([128, TX], f32)
            R = singles.tile([128, TY], f32)
            garbage = singles.tile([128, 512], bf16)
            nc.vector.memset(garbage[:], 0)

            # ---- input DMAs: x shard per-tile (fast head), y groups
            for t in range(TX):
                nc.sync.dma_start(
                    out=xnat[:, t, :], in_=xs_d[128 * t:128 * (t + 1), :])
            for g in range(TY // 8):
                nc.sync.dma_start(
                    out=yT[:, 8 * g:8 * (g + 1), :],
                    in_=ybt_d[:, 1024 * g:1024 * (g + 1)]
                    .rearrange("p (a b) -> p a b", b=128))

            # ---- PE warm-up fillers: flip HAM to 2.4 GHz during x-prep
            with tc.tile_pool(name="wpsum", bufs=1, space="PSUM") as wpsum:
                wp = wpsum.tile([128, 512], f32, tag="wp")
                for _ in range(12):
                    nc.tensor.matmul(wp[:], ident[:], garbage[:])

            # ---- x norms + scale to bf16 (DVE)
            for t in range(TX):
                nc.vector.scalar_tensor_tensor(
                    out=scr.tile([128, 128], f32, tag='sd', name='sd')[:],
                    in0=xnat[:, t, :], scalar=1.0, in1=xnat[:, t, :],
                    op0=ALU.mult, op1=ALU.mult, accum_out=nx2[:, t:t + 1])
            nc.vector.reciprocal(t1x[:], nx2[:])
            nc.scalar.sqrt(rnx[:], t1x[:])   # 1/||x_r||
            for t in range(TX):
                nc.vector.tensor_scalar(
                    out=xhat[:, t, :], in0=xnat[:, t, :],
                    scalar1=rnx[:, t:t + 1], scalar2=None, op0=ALU.mult)

            # ---- x transpose on PE (bf16), DVE copyback
            flat = xhatT[:].rearrange("p a b -> p (a b)")
            with tc.tile_pool(name="tpsum", bufs=1, space="PSUM") as tpsum:
                ptx = tpsum.tile([128, TX * 128], bf16, tag="tp")
                for t in range(TX):
                    nc.tensor.transpose(ptx[:, 128 * t:128 * (t + 1)],
                                        xhat[:, t, :], ident[:])
                nc.vector.tensor_copy(out=flat, in_=ptx[:])

            # ---- main: per j-block bf16 matmuls (fp32 PSUM, 2 banks,
            # bufs=4) + one relu+accum per block, ACT / DVE split
            with tc.tile_pool(name="mpsum", bufs=4, space="PSUM") as mpsum:
                for t in range(TY):
                    ps = mpsum.tile([128, 1024], f32, tag="mp")
                    lhsT = yT[:, t, :]
                    nc.tensor.matmul(ps[:, 0:512], lhsT, flat[:, 0:512])
                    nc.tensor.matmul(ps[:, 512:1024], lhsT, flat[:, 512:1024])
                    if ASSIGN[t] == "act":
                        nc.scalar.activation(
                            ps[:], ps[:], AF.Relu, accum_out=R[:, t:t + 1])
                    else:
                        nc.vector.tensor_scalar(
                            out=ps[:], in0=ps[:], scalar1=0.0, scalar2=None,
                            op0=ALU.max, op1=ALU.add,
                            accum_out=R[:, t:t + 1])

            # ---- output (two halves so the first can go out early)
            nc.sync.dma_start(out=out_d[:, :TY // 2], in_=R[:, :TY // 2])
            nc.sync.dma_start(out=out_d[:, TY // 2:], in_=R[:, TY // 2:])

    nc.compile()
    _CACHE["nc"] = nc
    return nc


def _in_maps(x, y):
    yb = y.astype(ml_dtypes.bfloat16)
    ybt = np.ascontiguousarray(yb.T)          # [D, N]
    maps = []
    for c in range(NCORES):
        sl = slice(SH * c, SH * (c + 1))
        maps.append({"xs": np.ascontiguousarray(x[sl]), "ybt": ybt})
    return maps


def _combine(results, x, y):
    x64 = x.astype(np.float64)
    y64 = y.astype(np.float64)
    ny = np.sqrt((y64 ** 2).sum(axis=1))
    rny = 1.0 / np.maximum(ny, 1e-8)          # [N]
    rny_pt = rny.reshape(TY, 128).T           # [128, TY], j = 128t + p
    total = 0.0
    for c in range(NCORES):
        R = results[c]["out"].astype(np.float64)      # [128, TY]
        total += (R * rny_pt).sum()
    # diagonal correction in fp64 on host (n of n^2 terms)
    nx = np.sqrt((x64 ** 2).sum(axis=1))
    sim_d = (x64 * y64).sum(axis=1) / np.maximum(nx * ny, 1e-8)
    total += (1.0 - sim_d - np.maximum(sim_d, 0.0)).sum()
    return np.float32(total / (float(N) * float(N)))


def _run(x, y, trace=False):
    nc = _build()
    res = run_bass_kernel_spmd(nc, _in_maps(x, y), list(range(NCORES)),
                               trace=trace)
    return _combine(res.results, x, y), res


def kernel(x, y):
    x = np.asarray(x, dtype=np.float32)
    y = np.asarray(y, dtype=np.float32)
    loss, _ = _run(x, y, trace=False)
    return loss
